# revision 17
# baseline (speedup 1.0000x reference)
"""Trainium2 Bass kernel for nn_MultiHeadAttention (B=8, S=1024, D=768, H=12).

Sharding: data-parallel over batch — one batch element per NeuronCore (8 cores).
No collectives needed; gather is a host-side stack.

Per-core layout strategy (all activations feature-major / "transposed"):
  inputs (host-prepped): xqT/xkT/xvT (D,S); WqT/WkT (D,D); WvT_pad (D, 12*65)
  with zero columns at each head's slot 64; WoT (D,D); biases.
  - QT[do,s] = WqT.T @ xqT + bq   (bias per-partition, fused in eviction)
  - KT[do,s] = WkT.T @ xkT + bk
  - V[t,dpad] = xvT.T @ WvT_pad   (natural layout, 65-wide head slots; the
    ones column per head makes attn@V also produce the softmax denominator.
    The value bias bv is folded into the output bias on the host:
    bo_eff = bo + Wo @ bv, exact because softmax rows sum to 1.)
  - per head pair j (heads 2j at partitions 0:64 of tile j, 2j+1 at 64:128):
      scoresT[t,s] = KT_h.T @ QT_h  (row-packed matmul pair, K=64 each)
      E = exp(SCALE * scoresT)      (ScalarE, PSUM->SBUF, both heads per op)
      acc[0:65,s] += V_aug.T @ E    (row 64 = softmax denominator Z)
      OHT_h = acc[0:64] * (1/Z)     (DVE mul with gpsimd partition_broadcast)
  - O[s,do] = OHT.T @ WoT + bo_eff (bias via K=1 rank-1 matmul)

All matmuls run as float32r (full-rate PE mode, ~1.6e-4 relative error).
"""
import sys

sys.path.insert(0, "/opt/trn_rl_repo")

import numpy as np

import concourse.bacc as bacc
import concourse.tile as tile
from concourse import mybir
from concourse.bass_utils import run_bass_kernel_spmd

B, S, D, H = 8, 1024, 768, 12
DH = D // H                       # 64
NP = H // 2                       # 6 head pairs == D/128 tiles
DVP = H * (DH + 1)                # 780: V padded width (65 per head)
SCALE = 1.0 / np.sqrt(np.float32(D))
NT = S // 128                     # 8 seq tiles of 128
ND = D // 128                     # 6 feature tiles of 128

F32 = mybir.dt.float32
F32R = mybir.dt.float32r
Exp = mybir.ActivationFunctionType.Exp

_CACHE = {}


def _build_nc(debug_outputs=False, loop_n=1):
    nc = bacc.Bacc("TRN2", target_bir_lowering=False, debug=False)

    d = {}
    for name, shape in [
        ("xqt", (D, S)), ("xkt", (D, S)), ("xvt", (D, S)),
        ("wqt", (D, D)), ("wkt", (D, D)), ("wvtp", (D, DVP)), ("wot", (D, D)),
        ("bqc", (128, ND)), ("bkc", (128, ND)), ("bor", (1, D)),
    ]:
        d[name] = nc.dram_tensor(name, shape, F32, kind="ExternalInput").ap()
    out_d = nc.dram_tensor("out", (S, D), F32, kind="ExternalOutput").ap()
    dbg = None
    if debug_outputs:
        dbg = {}
        for name, shape in [("dqt", (D, S)), ("dkt", (D, S)),
                            ("dv", (S, DVP)), ("doht", (D, S))]:
            dbg[name] = nc.dram_tensor(
                name, shape, F32, kind="ExternalOutput").ap()

    with tile.TileContext(nc) as tc:
        for _ in range(loop_n):
            _emit(nc, tc, d, out_d, dbg)
    nc.compile()
    return nc


def _emit(nc, tc, d, out_d, dbg=None):
    import contextlib

    ctx = contextlib.ExitStack()
    with ctx:
        w_pool = ctx.enter_context(tc.tile_pool(name="w", bufs=9))
        x_pool = ctx.enter_context(tc.tile_pool(name="x", bufs=9))
        qk_pool = ctx.enter_context(tc.tile_pool(name="qk", bufs=12))
        v_pool = ctx.enter_context(tc.tile_pool(name="v", bufs=8))
        e_pool = ctx.enter_context(tc.tile_pool(name="e", bufs=3))
        oht_pool = ctx.enter_context(tc.tile_pool(name="oht", bufs=6))
        o_pool = ctx.enter_context(tc.tile_pool(name="o", bufs=2))
        r_pool = ctx.enter_context(tc.tile_pool(name="r", bufs=2))
        rb_pool = ctx.enter_context(tc.tile_pool(name="rb", bufs=2))
        const_pool = ctx.enter_context(tc.tile_pool(name="const", bufs=1))
        ps = ctx.enter_context(tc.tile_pool(name="ps", bufs=3, space="PSUM"))
        ps_acc = ctx.enter_context(
            tc.tile_pool(name="ps_acc", bufs=2, space="PSUM"))

        # ---- constants ----
        bq_t = const_pool.tile([128, ND], F32, name="bq_t")
        bk_t = const_pool.tile([128, ND], F32, name="bk_t")
        bo_row = const_pool.tile([1, D], F32, name="bo_row")
        bo_bc = const_pool.tile([128, D], F32, name="bo_bc")
        nc.gpsimd.dma_start(bq_t[:], d["bqc"][:])
        nc.gpsimd.dma_start(bk_t[:], d["bkc"][:])
        nc.sync.dma_start(bo_row[:], d["bor"][:])
        nc.gpsimd.partition_broadcast(bo_bc[:], bo_row[:])

        def load_w(key, width):
            tiles = []
            for i in range(ND):
                t = w_pool.tile([128, width], F32R, name=f"{key}{i}", tag="w")
                nc.sync.dma_start(
                    t[:], d[key][i * 128:(i + 1) * 128, :].bitcast(F32R))
                tiles.append(t)
            return tiles

        def load_x(key):
            tiles = []
            for i in range(ND):
                t = x_pool.tile([128, S], F32R, name=f"{key}{i}", tag="x")
                nc.sync.dma_start(
                    t[:], d[key][i * 128:(i + 1) * 128, :].bitcast(F32R))
                tiles.append(t)
            return tiles

        def load_wx(wkey, wwidth, xkey):
            wt, xt = [], []
            for i in range(ND):
                w = w_pool.tile([128, wwidth], F32R, name=f"{wkey}{i}",
                                tag="w")
                nc.sync.dma_start(
                    w[:], d[wkey][i * 128:(i + 1) * 128, :].bitcast(F32R))
                x = x_pool.tile([128, S], F32R, name=f"{xkey}{i}", tag="x")
                nc.sync.dma_start(
                    x[:], d[xkey][i * 128:(i + 1) * 128, :].bitcast(F32R))
                wt.append(w)
                xt.append(x)
            return wt, xt

        # ---- V projection first ----
        wv, xv = load_wx("wvtp", DVP, "xvt")
        v_tiles = []
        for tb in range(NT):
            ps_v = ps.tile([128, DVP], F32, name=f"vps{tb}", tag="ps")
            for di in range(ND):
                st, sp = di == 0, di == ND - 1
                lhs = xv[di][:, tb * 128:(tb + 1) * 128]
                nc.tensor.matmul(ps_v[:, 0:512], lhs, wv[di][:, 0:512],
                                 start=st, stop=sp)
                nc.tensor.matmul(ps_v[:, 512:DVP], lhs, wv[di][:, 512:DVP],
                                 start=st, stop=sp)
            vt = v_pool.tile([128, DVP], F32R, name=f"v{tb}", tag="v")
            nc.vector.tensor_copy(vt[:], ps_v[:])
            # ones columns (head slot 64) for the denominator trick
            v3 = vt[:].bitcast(F32).rearrange("p (h e) -> p h e", e=DH + 1)
            nc.vector.memset(v3[:, :, DH:DH + 1], 1.0)
            v_tiles.append(vt)
            if dbg is not None:
                nc.sync.dma_start(
                    dbg["dv"][tb * 128:(tb + 1) * 128, :], vt[:].bitcast(F32))

        def proj_qk(which, w_t, x_t, b_t, p):
            ps_p = ps.tile([128, S], F32, name=f"{which}ps{p}", tag="ps")
            for di in range(ND):
                st, sp = di == 0, di == ND - 1
                lhs = w_t[di][:, p * 128:(p + 1) * 128]
                nc.tensor.matmul(ps_p[:, 0:512], lhs, x_t[di][:, 0:512],
                                 start=st, stop=sp)
                nc.tensor.matmul(ps_p[:, 512:S], lhs, x_t[di][:, 512:S],
                                 start=st, stop=sp)
            ot = qk_pool.tile([128, S], F32R, name=f"{which}t{p}", tag="qk")
            nc.vector.tensor_scalar_add(ot[:], ps_p[:], b_t[:, p:p + 1])
            if dbg is not None:
                nc.sync.dma_start(
                    dbg["dqt" if which == "q" else "dkt"]
                    [p * 128:(p + 1) * 128, :], ot[:].bitcast(F32))
            return ot

        # ---- all Q projections ----
        wq, xq = load_wx("wqt", D, "xqt")
        qt_tiles = [proj_qk("q", wq, xq, bq_t, p) for p in range(NP)]

        # ---- K projections, interleaved with strip-0 attention per pair ----
        wk, xk = load_wx("wkt", D, "xkt")
        kt_tiles = []

        # ---- attention + output projection ----
        oht_tiles = [
            oht_pool.tile([128, S], F32R, name=f"oht{p}", tag="oht")
            for p in range(NP)
        ]

        def o_proj(stt):
            po = ps.tile([128, D], F32, name=f"ops{stt}", tag="ps")
            ssl = slice(stt * 128, (stt + 1) * 128)
            for di in range(ND):
                nc.tensor.matmul(po[:, 0:512], oht_tiles[di][:, ssl],
                                 wo[di][:, 0:512],
                                 start=di == 0, stop=di == ND - 1)
                nc.tensor.matmul(po[:, 512:768], oht_tiles[di][:, ssl],
                                 wo[di][:, 512:768],
                                 start=di == 0, stop=di == ND - 1)
            o_t = o_pool.tile([128, D], F32, name=f"o{stt}", tag="o")
            nc.vector.tensor_add(o_t[:], po[:], bo_bc[:])
            nc.sync.dma_start(out_d[ssl, :], o_t[:])

        def attention(p, strip):
            sl = slice(strip * 512, strip * 512 + 512)
            acc0 = ps_acc.tile([65, 512], F32, name=f"acc0_{p}_{strip}",
                               tag="acc")
            acc1 = ps_acc.tile([65, 512], F32, name=f"acc1_{p}_{strip}",
                               tag="acc")
            for tb in range(NT):
                sc = ps.tile([128, 1024], F32,
                             name=f"sc{p}_{strip}_{tb}", tag="ps")
                tsl = slice(tb * 128, (tb + 1) * 128)
                nc.tensor.matmul(sc[:, 0:512], kt_tiles[p][0:64, tsl],
                                 qt_tiles[p][0:64, sl],
                                 start=True, stop=True)
                nc.tensor.matmul(sc[:, 512:1024], kt_tiles[p][64:128, tsl],
                                 qt_tiles[p][64:128, sl],
                                 start=True, stop=True)
                et = e_pool.tile([128, 1024], F32R,
                                 name=f"e{p}_{strip}_{tb}", tag="e")
                nc.scalar.activation(et[:], sc[:], Exp, scale=float(SCALE))
                st, sp = tb == 0, tb == NT - 1
                vt = v_tiles[tb]
                c0 = p * 2 * (DH + 1)
                nc.tensor.matmul(acc0[:], vt[:, c0:c0 + DH + 1],
                                 et[:, 0:512], start=st, stop=sp)
                nc.tensor.matmul(acc1[:], vt[:, c0 + DH + 1:c0 + 2 * (DH + 1)],
                                 et[:, 512:1024], start=st, stop=sp)
            for h01, acc in ((0, acc0), (1, acc1)):
                r = r_pool.tile([1, 512], F32, name=f"r{p}{strip}{h01}",
                                tag="r")
                nc.vector.reciprocal(r[:], acc[DH:DH + 1, :])
                rb = rb_pool.tile([64, 512], F32,
                                  name=f"rb{p}{strip}{h01}", tag="rb")
                nc.gpsimd.partition_broadcast(rb[:], r[:])
                nc.vector.tensor_mul(
                    oht_tiles[p][h01 * 64:(h01 + 1) * 64, sl],
                    acc[0:64, :], rb[:])

        # strip 0: emit KT(p+1) BEFORE attention(p) so the scheduler can
        # hide the next K projection under the current pair's exp stream
        kt_tiles.append(proj_qk("k", wk, xk, bk_t, 0))
        for p in range(1, NP):
            kt_tiles.append(proj_qk("k", wk, xk, bk_t, p))
            attention(p - 1, 0)
        attention(NP - 1, 0)

        wo = load_w("wot", D)

        # strip 1 attention, strip-0 output projections emitted ahead of the
        # attention block they should overlap with
        for p in range(NP):
            if p < 4:
                o_proj(p)
            attention(p, 1)
        for stt in range(4, 8):
            o_proj(stt)
        if dbg is not None:
            for p in range(NP):
                nc.sync.dma_start(
                    dbg["doht"][p * 128:(p + 1) * 128, :],
                    oht_tiles[p][:].bitcast(F32))


def _get_nc():
    if "nc" not in _CACHE:
        _CACHE["nc"] = _build_nc()
    return _CACHE["nc"]


def kernel(queries, keys, values, Wq, bq, Wk, bk, Wv, bv, Wo, bo):
    queries = np.ascontiguousarray(queries, dtype=np.float32)
    keys = np.ascontiguousarray(keys, dtype=np.float32)
    values = np.ascontiguousarray(values, dtype=np.float32)

    wqt = np.ascontiguousarray(np.asarray(Wq, np.float32).T)
    wkt = np.ascontiguousarray(np.asarray(Wk, np.float32).T)
    wot = np.ascontiguousarray(np.asarray(Wo, np.float32).T)
    wvt = np.asarray(Wv, np.float32).T              # (D, D) = (di, do)
    wvtp = np.zeros((D, DVP), np.float32)
    for h in range(H):
        wvtp[:, h * (DH + 1):h * (DH + 1) + DH] = \
            wvt[:, h * DH:(h + 1) * DH]
    bo_eff = (np.asarray(bo, np.float32)
              + np.asarray(Wo, np.float32) @ np.asarray(bv, np.float32))
    bqc = np.ascontiguousarray(
        np.asarray(bq, np.float32).reshape(ND, 128).T)
    bkc = np.ascontiguousarray(
        np.asarray(bk, np.float32).reshape(ND, 128).T)
    bor = np.ascontiguousarray(bo_eff.reshape(1, D))

    shared = {
        "wqt": wqt, "wkt": wkt, "wvtp": wvtp, "wot": wot,
        "bqc": bqc, "bkc": bkc, "bor": bor,
    }
    in_maps = []
    for b in range(B):
        in_maps.append({
            "xqt": np.ascontiguousarray(queries[b].T),
            "xkt": np.ascontiguousarray(keys[b].T),
            "xvt": np.ascontiguousarray(values[b].T),
            **shared,
        })

    nc = _get_nc()
    res = run_bass_kernel_spmd(nc, in_maps, core_ids=list(range(B)))
    return np.stack([res.results[b]["out"] for b in range(B)], axis=0)


def run_traced(inputs, tmpdir=None):
    """Profiled single run; returns BassKernelResults with exec_time_ns."""
    queries = np.ascontiguousarray(inputs["queries"], dtype=np.float32)
    keys = np.ascontiguousarray(inputs["keys"], dtype=np.float32)
    values = np.ascontiguousarray(inputs["values"], dtype=np.float32)
    Wq, bq = inputs["Wq"], inputs["bq"]
    Wk, bk = inputs["Wk"], inputs["bk"]
    Wv, bv = inputs["Wv"], inputs["bv"]
    Wo, bo = inputs["Wo"], inputs["bo"]
    wvt = np.asarray(Wv, np.float32).T
    wvtp = np.zeros((D, DVP), np.float32)
    for h in range(H):
        wvtp[:, h * (DH + 1):h * (DH + 1) + DH] = wvt[:, h * DH:(h + 1) * DH]
    bo_eff = (np.asarray(bo, np.float32)
              + np.asarray(Wo, np.float32) @ np.asarray(bv, np.float32))
    shared = {
        "wqt": np.ascontiguousarray(np.asarray(Wq, np.float32).T),
        "wkt": np.ascontiguousarray(np.asarray(Wk, np.float32).T),
        "wvtp": wvtp,
        "wot": np.ascontiguousarray(np.asarray(Wo, np.float32).T),
        "bqc": np.ascontiguousarray(np.asarray(bq, np.float32).reshape(ND, 128).T),
        "bkc": np.ascontiguousarray(np.asarray(bk, np.float32).reshape(ND, 128).T),
        "bor": np.ascontiguousarray(bo_eff.reshape(1, D)),
    }
    in_maps = [
        {"xqt": np.ascontiguousarray(queries[b].T),
         "xkt": np.ascontiguousarray(keys[b].T),
         "xvt": np.ascontiguousarray(values[b].T), **shared}
        for b in range(B)
    ]
    nc = _get_nc()
    return run_bass_kernel_spmd(nc, in_maps, core_ids=list(range(B)),
                                trace=True, tmpdir=tmpdir)


# revision 18
# speedup vs baseline: 1.0868x; 1.0868x over previous
"""Trainium2 Bass kernel for nn_MultiHeadAttention (B=8, S=1024, D=768, H=12).

Sharding: data-parallel over batch — one batch element per NeuronCore (8 cores).
No collectives needed; gather is a host-side stack.

Per-core layout strategy (all activations feature-major / "transposed"):
  inputs (host-prepped): xqT/xkT/xvT (D,S); WqT/WkT (D,D); WvT_pad (D, 12*65)
  with zero columns at each head's slot 64; WoT (D,D); biases.
  - QT[do,s] = WqT.T @ xqT + bq   (bias per-partition, fused in eviction)
  - KT[do,s] = WkT.T @ xkT + bk
  - V[t,dpad] = xvT.T @ WvT_pad   (natural layout, 65-wide head slots; the
    ones column per head makes attn@V also produce the softmax denominator.
    The value bias bv is folded into the output bias on the host:
    bo_eff = bo + Wo @ bv, exact because softmax rows sum to 1.)
  - per head pair j (heads 2j at partitions 0:64 of tile j, 2j+1 at 64:128):
      scoresT[t,s] = KT_h.T @ QT_h  (row-packed matmul pair, K=64 each)
      E = exp(SCALE * scoresT)      (ScalarE, PSUM->SBUF, both heads per op)
      acc[0:65,s] += V_aug.T @ E    (row 64 = softmax denominator Z)
      OHT_h = acc[0:64] * (1/Z)     (DVE mul with gpsimd partition_broadcast)
  - O[s,do] = OHT.T @ WoT + bo_eff (bias via K=1 rank-1 matmul)

All matmuls run as float32r (full-rate PE mode, ~1.6e-4 relative error).
"""
import sys

sys.path.insert(0, "/opt/trn_rl_repo")

import numpy as np

import concourse.bacc as bacc
import concourse.tile as tile
from concourse import mybir
from concourse.bass_utils import run_bass_kernel_spmd

B, S, D, H = 8, 1024, 768, 12
DH = D // H                       # 64
NP = H // 2                       # 6 head pairs == D/128 tiles
DVP = H * (DH + 1)                # 780: V padded width (65 per head)
SCALE = 1.0 / np.sqrt(np.float32(D))
NT = S // 128                     # 8 seq tiles of 128
ND = D // 128                     # 6 feature tiles of 128

F32 = mybir.dt.float32
F32R = mybir.dt.float32r
Exp = mybir.ActivationFunctionType.Exp

_CACHE = {}


def _build_nc(debug_outputs=False, loop_n=1):
    nc = bacc.Bacc("TRN2", target_bir_lowering=False, debug=False)

    d = {}
    for name, shape in [
        ("xqt", (D, S)), ("xkt", (D, S)), ("xvt", (D, S)),
        ("wqt", (D, D)), ("wkt", (D, D)), ("wvtp", (D, DVP)), ("wot", (D, D)),
        ("bqc", (128, ND)), ("bkc", (128, ND)), ("bor", (1, D)),
    ]:
        d[name] = nc.dram_tensor(name, shape, F32, kind="ExternalInput").ap()
    out_d = nc.dram_tensor("out", (S, D), F32, kind="ExternalOutput").ap()
    dbg = None
    if debug_outputs:
        dbg = {}
        for name, shape in [("dqt", (D, S)), ("dkt", (D, S)),
                            ("dv", (S, DVP)), ("doht", (D, S))]:
            dbg[name] = nc.dram_tensor(
                name, shape, F32, kind="ExternalOutput").ap()

    with tile.TileContext(nc) as tc:
        for _ in range(loop_n):
            _emit(nc, tc, d, out_d, dbg)
    nc.compile()
    return nc


def _emit(nc, tc, d, out_d, dbg=None):
    import contextlib

    ctx = contextlib.ExitStack()
    with ctx:
        w_pool = ctx.enter_context(tc.tile_pool(name="w", bufs=9))
        x_pool = ctx.enter_context(tc.tile_pool(name="x", bufs=9))
        qk_pool = ctx.enter_context(tc.tile_pool(name="qk", bufs=12))
        v_pool = ctx.enter_context(tc.tile_pool(name="v", bufs=8))
        e_pool = ctx.enter_context(tc.tile_pool(name="e", bufs=3))
        oht_pool = ctx.enter_context(tc.tile_pool(name="oht", bufs=6))
        o_pool = ctx.enter_context(tc.tile_pool(name="o", bufs=2))
        r_pool = ctx.enter_context(tc.tile_pool(name="r", bufs=2))
        rb_pool = ctx.enter_context(tc.tile_pool(name="rb", bufs=2))
        const_pool = ctx.enter_context(tc.tile_pool(name="const", bufs=1))
        ps = ctx.enter_context(tc.tile_pool(name="ps", bufs=3, space="PSUM"))
        ps_acc = ctx.enter_context(
            tc.tile_pool(name="ps_acc", bufs=2, space="PSUM"))

        # ---- constants ----
        bq_t = const_pool.tile([128, ND], F32, name="bq_t")
        bk_t = const_pool.tile([128, ND], F32, name="bk_t")
        bo_row = const_pool.tile([1, D], F32, name="bo_row")
        bo_bc = const_pool.tile([128, D], F32, name="bo_bc")
        nc.gpsimd.dma_start(bq_t[:], d["bqc"][:])
        nc.gpsimd.dma_start(bk_t[:], d["bkc"][:])
        nc.sync.dma_start(bo_row[:], d["bor"][:])
        nc.gpsimd.partition_broadcast(bo_bc[:], bo_row[:])

        def load_w(key, width):
            tiles = []
            for i in range(ND):
                t = w_pool.tile([128, width], F32R, name=f"{key}{i}", tag="w")
                nc.sync.dma_start(
                    t[:], d[key][i * 128:(i + 1) * 128, :].bitcast(F32R))
                tiles.append(t)
            return tiles

        def load_x(key):
            tiles = []
            for i in range(ND):
                t = x_pool.tile([128, S], F32R, name=f"{key}{i}", tag="x")
                nc.sync.dma_start(
                    t[:], d[key][i * 128:(i + 1) * 128, :].bitcast(F32R))
                tiles.append(t)
            return tiles

        def load_wx(wkey, wwidth, xkey):
            wt, xt = [], []
            for i in range(ND):
                w = w_pool.tile([128, wwidth], F32R, name=f"{wkey}{i}",
                                tag="w")
                nc.sync.dma_start(
                    w[:], d[wkey][i * 128:(i + 1) * 128, :].bitcast(F32R))
                x = x_pool.tile([128, S], F32R, name=f"{xkey}{i}", tag="x")
                nc.scalar.dma_start(
                    x[:], d[xkey][i * 128:(i + 1) * 128, :].bitcast(F32R))
                wt.append(w)
                xt.append(x)
            return wt, xt

        # ---- V projection first ----
        wv, xv = load_wx("wvtp", DVP, "xvt")
        v_tiles = []
        for tb in range(NT):
            ps_v = ps.tile([128, DVP], F32, name=f"vps{tb}", tag="ps")
            for di in range(ND):
                st, sp = di == 0, di == ND - 1
                lhs = xv[di][:, tb * 128:(tb + 1) * 128]
                nc.tensor.matmul(ps_v[:, 0:512], lhs, wv[di][:, 0:512],
                                 start=st, stop=sp)
                nc.tensor.matmul(ps_v[:, 512:DVP], lhs, wv[di][:, 512:DVP],
                                 start=st, stop=sp)
            vt = v_pool.tile([128, DVP], F32R, name=f"v{tb}", tag="v")
            nc.vector.tensor_copy(vt[:], ps_v[:])
            # ones columns (head slot 64) for the denominator trick
            v3 = vt[:].bitcast(F32).rearrange("p (h e) -> p h e", e=DH + 1)
            nc.vector.memset(v3[:, :, DH:DH + 1], 1.0)
            v_tiles.append(vt)
            if dbg is not None:
                nc.sync.dma_start(
                    dbg["dv"][tb * 128:(tb + 1) * 128, :], vt[:].bitcast(F32))

        def proj_qk(which, w_t, x_t, b_t, p):
            ps_p = ps.tile([128, S], F32, name=f"{which}ps{p}", tag="ps")
            for di in range(ND):
                st, sp = di == 0, di == ND - 1
                lhs = w_t[di][:, p * 128:(p + 1) * 128]
                nc.tensor.matmul(ps_p[:, 0:512], lhs, x_t[di][:, 0:512],
                                 start=st, stop=sp)
                nc.tensor.matmul(ps_p[:, 512:S], lhs, x_t[di][:, 512:S],
                                 start=st, stop=sp)
            ot = qk_pool.tile([128, S], F32R, name=f"{which}t{p}", tag="qk")
            nc.vector.tensor_scalar_add(ot[:], ps_p[:], b_t[:, p:p + 1])
            if dbg is not None:
                nc.sync.dma_start(
                    dbg["dqt" if which == "q" else "dkt"]
                    [p * 128:(p + 1) * 128, :], ot[:].bitcast(F32))
            return ot

        # ---- all Q projections ----
        wq, xq = load_wx("wqt", D, "xqt")
        qt_tiles = [proj_qk("q", wq, xq, bq_t, p) for p in range(NP)]

        # ---- K projections, interleaved with strip-0 attention per pair ----
        wk, xk = load_wx("wkt", D, "xkt")
        kt_tiles = []

        # ---- attention + output projection ----
        oht_tiles = [
            oht_pool.tile([128, S], F32R, name=f"oht{p}", tag="oht")
            for p in range(NP)
        ]

        def o_proj(stt):
            po = ps.tile([128, D], F32, name=f"ops{stt}", tag="ps")
            ssl = slice(stt * 128, (stt + 1) * 128)
            for di in range(ND):
                nc.tensor.matmul(po[:, 0:512], oht_tiles[di][:, ssl],
                                 wo[di][:, 0:512],
                                 start=di == 0, stop=di == ND - 1)
                nc.tensor.matmul(po[:, 512:768], oht_tiles[di][:, ssl],
                                 wo[di][:, 512:768],
                                 start=di == 0, stop=di == ND - 1)
            o_t = o_pool.tile([128, D], F32, name=f"o{stt}", tag="o")
            nc.vector.tensor_add(o_t[:], po[:], bo_bc[:])
            nc.sync.dma_start(out_d[ssl, :], o_t[:])

        def attention(p, strip):
            sl = slice(strip * 512, strip * 512 + 512)
            acc0 = ps_acc.tile([65, 512], F32, name=f"acc0_{p}_{strip}",
                               tag="acc")
            acc1 = ps_acc.tile([65, 512], F32, name=f"acc1_{p}_{strip}",
                               tag="acc")
            for tb in range(NT):
                sc = ps.tile([128, 1024], F32,
                             name=f"sc{p}_{strip}_{tb}", tag="ps")
                tsl = slice(tb * 128, (tb + 1) * 128)
                nc.tensor.matmul(sc[:, 0:512], kt_tiles[p][0:64, tsl],
                                 qt_tiles[p][0:64, sl],
                                 start=True, stop=True)
                nc.tensor.matmul(sc[:, 512:1024], kt_tiles[p][64:128, tsl],
                                 qt_tiles[p][64:128, sl],
                                 start=True, stop=True)
                et = e_pool.tile([128, 1024], F32R,
                                 name=f"e{p}_{strip}_{tb}", tag="e")
                nc.scalar.activation(et[:], sc[:], Exp, scale=float(SCALE))
                st, sp = tb == 0, tb == NT - 1
                vt = v_tiles[tb]
                c0 = p * 2 * (DH + 1)
                nc.tensor.matmul(acc0[:], vt[:, c0:c0 + DH + 1],
                                 et[:, 0:512], start=st, stop=sp)
                nc.tensor.matmul(acc1[:], vt[:, c0 + DH + 1:c0 + 2 * (DH + 1)],
                                 et[:, 512:1024], start=st, stop=sp)
            for h01, acc in ((0, acc0), (1, acc1)):
                r = r_pool.tile([1, 512], F32, name=f"r{p}{strip}{h01}",
                                tag="r")
                nc.vector.reciprocal(r[:], acc[DH:DH + 1, :])
                rb = rb_pool.tile([64, 512], F32,
                                  name=f"rb{p}{strip}{h01}", tag="rb")
                nc.gpsimd.partition_broadcast(rb[:], r[:])
                nc.vector.tensor_mul(
                    oht_tiles[p][h01 * 64:(h01 + 1) * 64, sl],
                    acc[0:64, :], rb[:])

        # strip 0: emit KT(p+1) BEFORE attention(p) so the scheduler can
        # hide the next K projection under the current pair's exp stream
        kt_tiles.append(proj_qk("k", wk, xk, bk_t, 0))
        for p in range(1, NP):
            kt_tiles.append(proj_qk("k", wk, xk, bk_t, p))
            attention(p - 1, 0)
        attention(NP - 1, 0)

        wo = load_w("wot", D)

        # strip 1 attention, strip-0 output projections emitted ahead of the
        # attention block they should overlap with
        for p in range(NP):
            if p < 4:
                o_proj(p)
            attention(p, 1)
        for stt in range(4, 8):
            o_proj(stt)
        if dbg is not None:
            for p in range(NP):
                nc.sync.dma_start(
                    dbg["doht"][p * 128:(p + 1) * 128, :],
                    oht_tiles[p][:].bitcast(F32))


def _get_nc():
    if "nc" not in _CACHE:
        _CACHE["nc"] = _build_nc()
    return _CACHE["nc"]


def kernel(queries, keys, values, Wq, bq, Wk, bk, Wv, bv, Wo, bo):
    queries = np.ascontiguousarray(queries, dtype=np.float32)
    keys = np.ascontiguousarray(keys, dtype=np.float32)
    values = np.ascontiguousarray(values, dtype=np.float32)

    wqt = np.ascontiguousarray(np.asarray(Wq, np.float32).T)
    wkt = np.ascontiguousarray(np.asarray(Wk, np.float32).T)
    wot = np.ascontiguousarray(np.asarray(Wo, np.float32).T)
    wvt = np.asarray(Wv, np.float32).T              # (D, D) = (di, do)
    wvtp = np.zeros((D, DVP), np.float32)
    for h in range(H):
        wvtp[:, h * (DH + 1):h * (DH + 1) + DH] = \
            wvt[:, h * DH:(h + 1) * DH]
    bo_eff = (np.asarray(bo, np.float32)
              + np.asarray(Wo, np.float32) @ np.asarray(bv, np.float32))
    bqc = np.ascontiguousarray(
        np.asarray(bq, np.float32).reshape(ND, 128).T)
    bkc = np.ascontiguousarray(
        np.asarray(bk, np.float32).reshape(ND, 128).T)
    bor = np.ascontiguousarray(bo_eff.reshape(1, D))

    shared = {
        "wqt": wqt, "wkt": wkt, "wvtp": wvtp, "wot": wot,
        "bqc": bqc, "bkc": bkc, "bor": bor,
    }
    in_maps = []
    for b in range(B):
        in_maps.append({
            "xqt": np.ascontiguousarray(queries[b].T),
            "xkt": np.ascontiguousarray(keys[b].T),
            "xvt": np.ascontiguousarray(values[b].T),
            **shared,
        })

    nc = _get_nc()
    res = run_bass_kernel_spmd(nc, in_maps, core_ids=list(range(B)))
    return np.stack([res.results[b]["out"] for b in range(B)], axis=0)


def run_traced(inputs, tmpdir=None):
    """Profiled single run; returns BassKernelResults with exec_time_ns."""
    queries = np.ascontiguousarray(inputs["queries"], dtype=np.float32)
    keys = np.ascontiguousarray(inputs["keys"], dtype=np.float32)
    values = np.ascontiguousarray(inputs["values"], dtype=np.float32)
    Wq, bq = inputs["Wq"], inputs["bq"]
    Wk, bk = inputs["Wk"], inputs["bk"]
    Wv, bv = inputs["Wv"], inputs["bv"]
    Wo, bo = inputs["Wo"], inputs["bo"]
    wvt = np.asarray(Wv, np.float32).T
    wvtp = np.zeros((D, DVP), np.float32)
    for h in range(H):
        wvtp[:, h * (DH + 1):h * (DH + 1) + DH] = wvt[:, h * DH:(h + 1) * DH]
    bo_eff = (np.asarray(bo, np.float32)
              + np.asarray(Wo, np.float32) @ np.asarray(bv, np.float32))
    shared = {
        "wqt": np.ascontiguousarray(np.asarray(Wq, np.float32).T),
        "wkt": np.ascontiguousarray(np.asarray(Wk, np.float32).T),
        "wvtp": wvtp,
        "wot": np.ascontiguousarray(np.asarray(Wo, np.float32).T),
        "bqc": np.ascontiguousarray(np.asarray(bq, np.float32).reshape(ND, 128).T),
        "bkc": np.ascontiguousarray(np.asarray(bk, np.float32).reshape(ND, 128).T),
        "bor": np.ascontiguousarray(bo_eff.reshape(1, D)),
    }
    in_maps = [
        {"xqt": np.ascontiguousarray(queries[b].T),
         "xkt": np.ascontiguousarray(keys[b].T),
         "xvt": np.ascontiguousarray(values[b].T), **shared}
        for b in range(B)
    ]
    nc = _get_nc()
    return run_bass_kernel_spmd(nc, in_maps, core_ids=list(range(B)),
                                trace=True, tmpdir=tmpdir)


# revision 19
# speedup vs baseline: 1.1207x; 1.0312x over previous
"""Trainium2 Bass kernel for nn_MultiHeadAttention (B=8, S=1024, D=768, H=12).

Sharding: data-parallel over batch — one batch element per NeuronCore (8 cores).
No collectives needed; gather is a host-side stack.

Per-core layout strategy (all activations feature-major / "transposed"):
  inputs (host-prepped): xqT/xkT/xvT (D,S); WqT/WkT (D,D); WvT_pad (D, 12*65)
  with zero columns at each head's slot 64; WoT (D,D); biases.
  - QT[do,s] = WqT.T @ xqT + bq   (bias per-partition, fused in eviction)
  - KT[do,s] = WkT.T @ xkT + bk
  - V[t,dpad] = xvT.T @ WvT_pad   (natural layout, 65-wide head slots; the
    ones column per head makes attn@V also produce the softmax denominator.
    The value bias bv is folded into the output bias on the host:
    bo_eff = bo + Wo @ bv, exact because softmax rows sum to 1.)
  - per head pair j (heads 2j at partitions 0:64 of tile j, 2j+1 at 64:128):
      scoresT[t,s] = KT_h.T @ QT_h  (row-packed matmul pair, K=64 each)
      E = exp(SCALE * scoresT)      (ScalarE, PSUM->SBUF, both heads per op)
      acc[0:65,s] += V_aug.T @ E    (row 64 = softmax denominator Z)
      OHT_h = acc[0:64] * (1/Z)     (DVE mul with gpsimd partition_broadcast)
  - O[s,do] = OHT.T @ WoT + bo_eff (bias via K=1 rank-1 matmul)

All matmuls run as float32r (full-rate PE mode, ~1.6e-4 relative error).
"""
import sys

sys.path.insert(0, "/opt/trn_rl_repo")

import numpy as np

import concourse.bacc as bacc
import concourse.tile as tile
from concourse import mybir
from concourse.bass_utils import run_bass_kernel_spmd

B, S, D, H = 8, 1024, 768, 12
DH = D // H                       # 64
NP = H // 2                       # 6 head pairs == D/128 tiles
DVP = H * (DH + 1)                # 780: V padded width (65 per head)
SCALE = 1.0 / np.sqrt(np.float32(D))
NT = S // 128                     # 8 seq tiles of 128
ND = D // 128                     # 6 feature tiles of 128

F32 = mybir.dt.float32
F32R = mybir.dt.float32r
Exp = mybir.ActivationFunctionType.Exp

_CACHE = {}


def _build_nc(debug_outputs=False, loop_n=1):
    nc = bacc.Bacc("TRN2", target_bir_lowering=False, debug=False)

    d = {}
    for name, shape in [
        ("xqt", (D, S)), ("xkt", (D, S)), ("xvt", (D, S)),
        ("wqt", (D, D)), ("wkt", (D, D)), ("wvtp", (D, DVP)), ("wot", (D, D)),
        ("bqc", (128, ND)), ("bkc", (128, ND)), ("bor", (1, D)),
    ]:
        d[name] = nc.dram_tensor(name, shape, F32, kind="ExternalInput").ap()
    out_d = nc.dram_tensor("out", (S, D), F32, kind="ExternalOutput").ap()
    dbg = None
    if debug_outputs:
        dbg = {}
        for name, shape in [("dqt", (D, S)), ("dkt", (D, S)),
                            ("dv", (S, DVP)), ("doht", (D, S))]:
            dbg[name] = nc.dram_tensor(
                name, shape, F32, kind="ExternalOutput").ap()

    with tile.TileContext(nc) as tc:
        for _ in range(loop_n):
            _emit(nc, tc, d, out_d, dbg)
    nc.compile()
    return nc


def _emit(nc, tc, d, out_d, dbg=None):
    import contextlib

    ctx = contextlib.ExitStack()
    with ctx:
        w_pool = ctx.enter_context(tc.tile_pool(name="w", bufs=9))
        x_pool = ctx.enter_context(tc.tile_pool(name="x", bufs=9))
        qk_pool = ctx.enter_context(tc.tile_pool(name="qk", bufs=12))
        v_pool = ctx.enter_context(tc.tile_pool(name="v", bufs=8))
        e_pool = ctx.enter_context(tc.tile_pool(name="e", bufs=3))
        oht_pool = ctx.enter_context(tc.tile_pool(name="oht", bufs=6))
        o_pool = ctx.enter_context(tc.tile_pool(name="o", bufs=2))
        r_pool = ctx.enter_context(tc.tile_pool(name="r", bufs=2))
        rb_pool = ctx.enter_context(tc.tile_pool(name="rb", bufs=2))
        const_pool = ctx.enter_context(tc.tile_pool(name="const", bufs=1))
        ps = ctx.enter_context(tc.tile_pool(name="ps", bufs=2, space="PSUM"))
        ps_acc = ctx.enter_context(
            tc.tile_pool(name="ps_acc", bufs=4, space="PSUM"))

        # ---- constants ----
        bq_t = const_pool.tile([128, ND], F32, name="bq_t")
        bk_t = const_pool.tile([128, ND], F32, name="bk_t")
        bo_row = const_pool.tile([1, D], F32, name="bo_row")
        bo_bc = const_pool.tile([128, D], F32, name="bo_bc")
        nc.gpsimd.dma_start(bq_t[:], d["bqc"][:])
        nc.gpsimd.dma_start(bk_t[:], d["bkc"][:])
        nc.sync.dma_start(bo_row[:], d["bor"][:])
        nc.gpsimd.partition_broadcast(bo_bc[:], bo_row[:])

        def load_w(key, width):
            tiles = []
            for i in range(ND):
                t = w_pool.tile([128, width], F32R, name=f"{key}{i}", tag="w")
                nc.sync.dma_start(
                    t[:], d[key][i * 128:(i + 1) * 128, :].bitcast(F32R))
                tiles.append(t)
            return tiles

        def load_x(key):
            tiles = []
            for i in range(ND):
                t = x_pool.tile([128, S], F32R, name=f"{key}{i}", tag="x")
                nc.sync.dma_start(
                    t[:], d[key][i * 128:(i + 1) * 128, :].bitcast(F32R))
                tiles.append(t)
            return tiles

        def load_wx(wkey, wwidth, xkey):
            wt, xt = [], []
            for i in range(ND):
                w = w_pool.tile([128, wwidth], F32R, name=f"{wkey}{i}",
                                tag="w")
                nc.sync.dma_start(
                    w[:], d[wkey][i * 128:(i + 1) * 128, :].bitcast(F32R))
                x = x_pool.tile([128, S], F32R, name=f"{xkey}{i}", tag="x")
                nc.scalar.dma_start(
                    x[:], d[xkey][i * 128:(i + 1) * 128, :].bitcast(F32R))
                wt.append(w)
                xt.append(x)
            return wt, xt

        # ---- V projection first ----
        wv, xv = load_wx("wvtp", DVP, "xvt")
        v_tiles = []
        for tb in range(NT):
            ps_v = ps.tile([128, DVP], F32, name=f"vps{tb}", tag="ps")
            for di in range(ND):
                st, sp = di == 0, di == ND - 1
                lhs = xv[di][:, tb * 128:(tb + 1) * 128]
                nc.tensor.matmul(ps_v[:, 0:512], lhs, wv[di][:, 0:512],
                                 start=st, stop=sp)
                nc.tensor.matmul(ps_v[:, 512:DVP], lhs, wv[di][:, 512:DVP],
                                 start=st, stop=sp)
            vt = v_pool.tile([128, DVP], F32R, name=f"v{tb}", tag="v")
            nc.vector.tensor_copy(vt[:], ps_v[:])
            # ones columns (head slot 64) for the denominator trick
            v3 = vt[:].bitcast(F32).rearrange("p (h e) -> p h e", e=DH + 1)
            nc.vector.memset(v3[:, :, DH:DH + 1], 1.0)
            v_tiles.append(vt)
            if dbg is not None:
                nc.sync.dma_start(
                    dbg["dv"][tb * 128:(tb + 1) * 128, :], vt[:].bitcast(F32))

        def proj_qk(which, w_t, x_t, b_t, p):
            ps_p = ps.tile([128, S], F32, name=f"{which}ps{p}", tag="ps")
            for di in range(ND):
                st, sp = di == 0, di == ND - 1
                lhs = w_t[di][:, p * 128:(p + 1) * 128]
                nc.tensor.matmul(ps_p[:, 0:512], lhs, x_t[di][:, 0:512],
                                 start=st, stop=sp)
                nc.tensor.matmul(ps_p[:, 512:S], lhs, x_t[di][:, 512:S],
                                 start=st, stop=sp)
            ot = qk_pool.tile([128, S], F32R, name=f"{which}t{p}", tag="qk")
            nc.vector.tensor_scalar_add(ot[:], ps_p[:], b_t[:, p:p + 1])
            if dbg is not None:
                nc.sync.dma_start(
                    dbg["dqt" if which == "q" else "dkt"]
                    [p * 128:(p + 1) * 128, :], ot[:].bitcast(F32))
            return ot

        # ---- all Q projections ----
        wq, xq = load_wx("wqt", D, "xqt")
        qt_tiles = [proj_qk("q", wq, xq, bq_t, p) for p in range(NP)]

        # ---- K projections, interleaved with strip-0 attention per pair ----
        wk, xk = load_wx("wkt", D, "xkt")
        kt_tiles = []

        # ---- attention + output projection ----
        oht_tiles = [
            oht_pool.tile([128, S], F32R, name=f"oht{p}", tag="oht")
            for p in range(NP)
        ]

        def o_proj(stt):
            po = ps.tile([128, D], F32, name=f"ops{stt}", tag="ps")
            ssl = slice(stt * 128, (stt + 1) * 128)
            for di in range(ND):
                nc.tensor.matmul(po[:, 0:512], oht_tiles[di][:, ssl],
                                 wo[di][:, 0:512],
                                 start=di == 0, stop=di == ND - 1)
                nc.tensor.matmul(po[:, 512:768], oht_tiles[di][:, ssl],
                                 wo[di][:, 512:768],
                                 start=di == 0, stop=di == ND - 1)
            o_t = o_pool.tile([128, D], F32, name=f"o{stt}", tag="o")
            nc.vector.tensor_add(o_t[:], po[:], bo_bc[:])
            nc.sync.dma_start(out_d[ssl, :], o_t[:])

        def attention(p, strip):
            sl = slice(strip * 512, strip * 512 + 512)
            acc0 = ps_acc.tile([65, 512], F32, name=f"acc0_{p}_{strip}",
                               tag="acc")
            acc1 = ps_acc.tile([65, 512], F32, name=f"acc1_{p}_{strip}",
                               tag="acc")
            for tb in range(NT):
                sc = ps.tile([128, 1024], F32,
                             name=f"sc{p}_{strip}_{tb}", tag="ps")
                tsl = slice(tb * 128, (tb + 1) * 128)
                nc.tensor.matmul(sc[:, 0:512], kt_tiles[p][0:64, tsl],
                                 qt_tiles[p][0:64, sl],
                                 start=True, stop=True)
                nc.tensor.matmul(sc[:, 512:1024], kt_tiles[p][64:128, tsl],
                                 qt_tiles[p][64:128, sl],
                                 start=True, stop=True)
                et = e_pool.tile([128, 1024], F32R,
                                 name=f"e{p}_{strip}_{tb}", tag="e")
                nc.scalar.activation(et[:], sc[:], Exp, scale=float(SCALE))
                st, sp = tb == 0, tb == NT - 1
                vt = v_tiles[tb]
                c0 = p * 2 * (DH + 1)
                nc.tensor.matmul(acc0[:], vt[:, c0:c0 + DH + 1],
                                 et[:, 0:512], start=st, stop=sp)
                nc.tensor.matmul(acc1[:], vt[:, c0 + DH + 1:c0 + 2 * (DH + 1)],
                                 et[:, 512:1024], start=st, stop=sp)
            for h01, acc in ((0, acc0), (1, acc1)):
                r = r_pool.tile([1, 512], F32, name=f"r{p}{strip}{h01}",
                                tag="r")
                nc.vector.reciprocal(r[:], acc[DH:DH + 1, :])
                rb = rb_pool.tile([64, 512], F32,
                                  name=f"rb{p}{strip}{h01}", tag="rb")
                nc.gpsimd.partition_broadcast(rb[:], r[:])
                nc.vector.tensor_mul(
                    oht_tiles[p][h01 * 64:(h01 + 1) * 64, sl],
                    acc[0:64, :], rb[:])

        # strip 0: emit KT(p+1) BEFORE attention(p) so the scheduler can
        # hide the next K projection under the current pair's exp stream
        kt_tiles.append(proj_qk("k", wk, xk, bk_t, 0))
        for p in range(1, NP):
            kt_tiles.append(proj_qk("k", wk, xk, bk_t, p))
            attention(p - 1, 0)
        attention(NP - 1, 0)

        wo = load_w("wot", D)

        # strip 1 attention, strip-0 output projections emitted ahead of the
        # attention block they should overlap with
        for p in range(NP):
            if p < 4:
                o_proj(p)
            attention(p, 1)
        for stt in range(4, 8):
            o_proj(stt)
        if dbg is not None:
            for p in range(NP):
                nc.sync.dma_start(
                    dbg["doht"][p * 128:(p + 1) * 128, :],
                    oht_tiles[p][:].bitcast(F32))


def _get_nc():
    if "nc" not in _CACHE:
        _CACHE["nc"] = _build_nc()
    return _CACHE["nc"]


def kernel(queries, keys, values, Wq, bq, Wk, bk, Wv, bv, Wo, bo):
    queries = np.ascontiguousarray(queries, dtype=np.float32)
    keys = np.ascontiguousarray(keys, dtype=np.float32)
    values = np.ascontiguousarray(values, dtype=np.float32)

    wqt = np.ascontiguousarray(np.asarray(Wq, np.float32).T)
    wkt = np.ascontiguousarray(np.asarray(Wk, np.float32).T)
    wot = np.ascontiguousarray(np.asarray(Wo, np.float32).T)
    wvt = np.asarray(Wv, np.float32).T              # (D, D) = (di, do)
    wvtp = np.zeros((D, DVP), np.float32)
    for h in range(H):
        wvtp[:, h * (DH + 1):h * (DH + 1) + DH] = \
            wvt[:, h * DH:(h + 1) * DH]
    bo_eff = (np.asarray(bo, np.float32)
              + np.asarray(Wo, np.float32) @ np.asarray(bv, np.float32))
    bqc = np.ascontiguousarray(
        np.asarray(bq, np.float32).reshape(ND, 128).T)
    bkc = np.ascontiguousarray(
        np.asarray(bk, np.float32).reshape(ND, 128).T)
    bor = np.ascontiguousarray(bo_eff.reshape(1, D))

    shared = {
        "wqt": wqt, "wkt": wkt, "wvtp": wvtp, "wot": wot,
        "bqc": bqc, "bkc": bkc, "bor": bor,
    }
    in_maps = []
    for b in range(B):
        in_maps.append({
            "xqt": np.ascontiguousarray(queries[b].T),
            "xkt": np.ascontiguousarray(keys[b].T),
            "xvt": np.ascontiguousarray(values[b].T),
            **shared,
        })

    nc = _get_nc()
    res = run_bass_kernel_spmd(nc, in_maps, core_ids=list(range(B)))
    return np.stack([res.results[b]["out"] for b in range(B)], axis=0)


def run_traced(inputs, tmpdir=None):
    """Profiled single run; returns BassKernelResults with exec_time_ns."""
    queries = np.ascontiguousarray(inputs["queries"], dtype=np.float32)
    keys = np.ascontiguousarray(inputs["keys"], dtype=np.float32)
    values = np.ascontiguousarray(inputs["values"], dtype=np.float32)
    Wq, bq = inputs["Wq"], inputs["bq"]
    Wk, bk = inputs["Wk"], inputs["bk"]
    Wv, bv = inputs["Wv"], inputs["bv"]
    Wo, bo = inputs["Wo"], inputs["bo"]
    wvt = np.asarray(Wv, np.float32).T
    wvtp = np.zeros((D, DVP), np.float32)
    for h in range(H):
        wvtp[:, h * (DH + 1):h * (DH + 1) + DH] = wvt[:, h * DH:(h + 1) * DH]
    bo_eff = (np.asarray(bo, np.float32)
              + np.asarray(Wo, np.float32) @ np.asarray(bv, np.float32))
    shared = {
        "wqt": np.ascontiguousarray(np.asarray(Wq, np.float32).T),
        "wkt": np.ascontiguousarray(np.asarray(Wk, np.float32).T),
        "wvtp": wvtp,
        "wot": np.ascontiguousarray(np.asarray(Wo, np.float32).T),
        "bqc": np.ascontiguousarray(np.asarray(bq, np.float32).reshape(ND, 128).T),
        "bkc": np.ascontiguousarray(np.asarray(bk, np.float32).reshape(ND, 128).T),
        "bor": np.ascontiguousarray(bo_eff.reshape(1, D)),
    }
    in_maps = [
        {"xqt": np.ascontiguousarray(queries[b].T),
         "xkt": np.ascontiguousarray(keys[b].T),
         "xvt": np.ascontiguousarray(values[b].T), **shared}
        for b in range(B)
    ]
    nc = _get_nc()
    return run_bass_kernel_spmd(nc, in_maps, core_ids=list(range(B)),
                                trace=True, tmpdir=tmpdir)


# revision 20
# speedup vs baseline: 1.1435x; 1.0203x over previous
"""Trainium2 Bass kernel for nn_MultiHeadAttention (B=8, S=1024, D=768, H=12).

Sharding: data-parallel over batch — one batch element per NeuronCore (8 cores).
No collectives needed; gather is a host-side stack.

Per-core layout strategy (all activations feature-major / "transposed"):
  inputs (host-prepped): xqT/xkT/xvT (D,S); WqT/WkT (D,D); WvT_pad (D, 12*65)
  with zero columns at each head's slot 64; WoT (D,D); biases.
  - QT[do,s] = WqT.T @ xqT + bq   (bias per-partition, fused in eviction)
  - KT[do,s] = WkT.T @ xkT + bk
  - V[t,dpad] = xvT.T @ WvT_pad   (natural layout, 65-wide head slots; the
    ones column per head makes attn@V also produce the softmax denominator.
    The value bias bv is folded into the output bias on the host:
    bo_eff = bo + Wo @ bv, exact because softmax rows sum to 1.)
  - per head pair j (heads 2j at partitions 0:64 of tile j, 2j+1 at 64:128):
      scoresT[t,s] = KT_h.T @ QT_h  (row-packed matmul pair, K=64 each)
      E = exp(SCALE * scoresT)      (ScalarE, PSUM->SBUF, both heads per op)
      acc[0:65,s] += V_aug.T @ E    (row 64 = softmax denominator Z)
      OHT_h = acc[0:64] * (1/Z)     (DVE mul with gpsimd partition_broadcast)
  - O[s,do] = OHT.T @ WoT + bo_eff (bias via K=1 rank-1 matmul)

All matmuls run as float32r (full-rate PE mode, ~1.6e-4 relative error).
"""
import sys

sys.path.insert(0, "/opt/trn_rl_repo")

import numpy as np

import concourse.bacc as bacc
import concourse.tile as tile
from concourse import mybir
from concourse.bass_utils import run_bass_kernel_spmd

B, S, D, H = 8, 1024, 768, 12
DH = D // H                       # 64
NP = H // 2                       # 6 head pairs == D/128 tiles
DVP = H * (DH + 1)                # 780: V padded width (65 per head)
SCALE = 1.0 / np.sqrt(np.float32(D))
NT = S // 128                     # 8 seq tiles of 128
ND = D // 128                     # 6 feature tiles of 128

F32 = mybir.dt.float32
F32R = mybir.dt.float32r
Exp = mybir.ActivationFunctionType.Exp

_CACHE = {}


def _build_nc(debug_outputs=False, loop_n=1):
    nc = bacc.Bacc("TRN2", target_bir_lowering=False, debug=False)

    d = {}
    for name, shape in [
        ("xqt", (D, S)), ("xkt", (D, S)), ("xvt", (D, S)),
        ("wqt", (D, D)), ("wkt", (D, D)), ("wvtp", (D, DVP)), ("wot", (D, D)),
        ("bqc", (128, ND)), ("bkc", (128, ND)), ("bor", (1, D)),
    ]:
        d[name] = nc.dram_tensor(name, shape, F32, kind="ExternalInput").ap()
    out_d = nc.dram_tensor("out", (S, D), F32, kind="ExternalOutput").ap()
    dbg = None
    if debug_outputs:
        dbg = {}
        for name, shape in [("dqt", (D, S)), ("dkt", (D, S)),
                            ("dv", (S, DVP)), ("doht", (D, S))]:
            dbg[name] = nc.dram_tensor(
                name, shape, F32, kind="ExternalOutput").ap()

    with tile.TileContext(nc) as tc:
        for _ in range(loop_n):
            _emit(nc, tc, d, out_d, dbg)
    nc.compile()
    return nc


def _emit(nc, tc, d, out_d, dbg=None):
    import contextlib

    ctx = contextlib.ExitStack()
    with ctx:
        w_pool = ctx.enter_context(tc.tile_pool(name="w", bufs=9))
        x_pool = ctx.enter_context(tc.tile_pool(name="x", bufs=9))
        qk_pool = ctx.enter_context(tc.tile_pool(name="qk", bufs=12))
        v_pool = ctx.enter_context(tc.tile_pool(name="v", bufs=8))
        e_pool = ctx.enter_context(tc.tile_pool(name="e", bufs=4))
        oht_pool = ctx.enter_context(tc.tile_pool(name="oht", bufs=6))
        o_pool = ctx.enter_context(tc.tile_pool(name="o", bufs=2))
        r_pool = ctx.enter_context(tc.tile_pool(name="r", bufs=2))
        rb_pool = ctx.enter_context(tc.tile_pool(name="rb", bufs=2))
        const_pool = ctx.enter_context(tc.tile_pool(name="const", bufs=1))
        ps = ctx.enter_context(tc.tile_pool(name="ps", bufs=2, space="PSUM"))
        ps_acc = ctx.enter_context(
            tc.tile_pool(name="ps_acc", bufs=4, space="PSUM"))

        # ---- constants ----
        bq_t = const_pool.tile([128, ND], F32, name="bq_t")
        bk_t = const_pool.tile([128, ND], F32, name="bk_t")
        bo_row = const_pool.tile([1, D], F32, name="bo_row")
        bo_bc = const_pool.tile([128, D], F32, name="bo_bc")
        nc.gpsimd.dma_start(bq_t[:], d["bqc"][:])
        nc.gpsimd.dma_start(bk_t[:], d["bkc"][:])
        nc.sync.dma_start(bo_row[:], d["bor"][:])
        nc.gpsimd.partition_broadcast(bo_bc[:], bo_row[:])

        def load_w(key, width):
            tiles = []
            for i in range(ND):
                t = w_pool.tile([128, width], F32R, name=f"{key}{i}", tag="w")
                nc.sync.dma_start(
                    t[:], d[key][i * 128:(i + 1) * 128, :].bitcast(F32R))
                tiles.append(t)
            return tiles

        def load_x(key):
            tiles = []
            for i in range(ND):
                t = x_pool.tile([128, S], F32R, name=f"{key}{i}", tag="x")
                nc.sync.dma_start(
                    t[:], d[key][i * 128:(i + 1) * 128, :].bitcast(F32R))
                tiles.append(t)
            return tiles

        def load_wx(wkey, wwidth, xkey):
            wt, xt = [], []
            for i in range(ND):
                w = w_pool.tile([128, wwidth], F32R, name=f"{wkey}{i}",
                                tag="w")
                nc.sync.dma_start(
                    w[:], d[wkey][i * 128:(i + 1) * 128, :].bitcast(F32R))
                x = x_pool.tile([128, S], F32R, name=f"{xkey}{i}", tag="x")
                nc.scalar.dma_start(
                    x[:], d[xkey][i * 128:(i + 1) * 128, :].bitcast(F32R))
                wt.append(w)
                xt.append(x)
            return wt, xt

        # ---- V projection first ----
        wv, xv = load_wx("wvtp", DVP, "xvt")
        v_tiles = []
        for tb in range(NT):
            ps_v = ps.tile([128, DVP], F32, name=f"vps{tb}", tag="ps")
            for di in range(ND):
                st, sp = di == 0, di == ND - 1
                lhs = xv[di][:, tb * 128:(tb + 1) * 128]
                nc.tensor.matmul(ps_v[:, 0:512], lhs, wv[di][:, 0:512],
                                 start=st, stop=sp)
                nc.tensor.matmul(ps_v[:, 512:DVP], lhs, wv[di][:, 512:DVP],
                                 start=st, stop=sp)
            vt = v_pool.tile([128, DVP], F32R, name=f"v{tb}", tag="v")
            nc.vector.tensor_copy(vt[:], ps_v[:])
            # ones columns (head slot 64) for the denominator trick
            v3 = vt[:].bitcast(F32).rearrange("p (h e) -> p h e", e=DH + 1)
            nc.vector.memset(v3[:, :, DH:DH + 1], 1.0)
            v_tiles.append(vt)
            if dbg is not None:
                nc.sync.dma_start(
                    dbg["dv"][tb * 128:(tb + 1) * 128, :], vt[:].bitcast(F32))

        def proj_qk(which, w_t, x_t, b_t, p):
            ps_p = ps.tile([128, S], F32, name=f"{which}ps{p}", tag="ps")
            for di in range(ND):
                st, sp = di == 0, di == ND - 1
                lhs = w_t[di][:, p * 128:(p + 1) * 128]
                nc.tensor.matmul(ps_p[:, 0:512], lhs, x_t[di][:, 0:512],
                                 start=st, stop=sp)
                nc.tensor.matmul(ps_p[:, 512:S], lhs, x_t[di][:, 512:S],
                                 start=st, stop=sp)
            ot = qk_pool.tile([128, S], F32R, name=f"{which}t{p}", tag="qk")
            nc.vector.tensor_scalar_add(ot[:], ps_p[:], b_t[:, p:p + 1])
            if dbg is not None:
                nc.sync.dma_start(
                    dbg["dqt" if which == "q" else "dkt"]
                    [p * 128:(p + 1) * 128, :], ot[:].bitcast(F32))
            return ot

        # ---- all Q projections ----
        wq, xq = load_wx("wqt", D, "xqt")
        qt_tiles = [proj_qk("q", wq, xq, bq_t, p) for p in range(NP)]

        # ---- K projections, interleaved with strip-0 attention per pair ----
        wk, xk = load_wx("wkt", D, "xkt")
        kt_tiles = []

        # ---- attention + output projection ----
        oht_tiles = [
            oht_pool.tile([128, S], F32R, name=f"oht{p}", tag="oht")
            for p in range(NP)
        ]

        def o_proj(stt):
            po = ps.tile([128, D], F32, name=f"ops{stt}", tag="ps")
            ssl = slice(stt * 128, (stt + 1) * 128)
            for di in range(ND):
                nc.tensor.matmul(po[:, 0:512], oht_tiles[di][:, ssl],
                                 wo[di][:, 0:512],
                                 start=di == 0, stop=di == ND - 1)
                nc.tensor.matmul(po[:, 512:768], oht_tiles[di][:, ssl],
                                 wo[di][:, 512:768],
                                 start=di == 0, stop=di == ND - 1)
            o_t = o_pool.tile([128, D], F32, name=f"o{stt}", tag="o")
            nc.vector.tensor_add(o_t[:], po[:], bo_bc[:])
            nc.sync.dma_start(out_d[ssl, :], o_t[:])

        def attention(p, strip):
            sl = slice(strip * 512, strip * 512 + 512)
            acc0 = ps_acc.tile([65, 512], F32, name=f"acc0_{p}_{strip}",
                               tag="acc")
            acc1 = ps_acc.tile([65, 512], F32, name=f"acc1_{p}_{strip}",
                               tag="acc")
            for tb in range(NT):
                sc = ps.tile([128, 1024], F32,
                             name=f"sc{p}_{strip}_{tb}", tag="ps")
                tsl = slice(tb * 128, (tb + 1) * 128)
                nc.tensor.matmul(sc[:, 0:512], kt_tiles[p][0:64, tsl],
                                 qt_tiles[p][0:64, sl],
                                 start=True, stop=True)
                nc.tensor.matmul(sc[:, 512:1024], kt_tiles[p][64:128, tsl],
                                 qt_tiles[p][64:128, sl],
                                 start=True, stop=True)
                et = e_pool.tile([128, 1024], F32R,
                                 name=f"e{p}_{strip}_{tb}", tag="e")
                nc.scalar.activation(et[:], sc[:], Exp, scale=float(SCALE))
                st, sp = tb == 0, tb == NT - 1
                vt = v_tiles[tb]
                c0 = p * 2 * (DH + 1)
                nc.tensor.matmul(acc0[:], vt[:, c0:c0 + DH + 1],
                                 et[:, 0:512], start=st, stop=sp)
                nc.tensor.matmul(acc1[:], vt[:, c0 + DH + 1:c0 + 2 * (DH + 1)],
                                 et[:, 512:1024], start=st, stop=sp)
            for h01, acc in ((0, acc0), (1, acc1)):
                r = r_pool.tile([1, 512], F32, name=f"r{p}{strip}{h01}",
                                tag="r")
                nc.vector.reciprocal(r[:], acc[DH:DH + 1, :])
                rb = rb_pool.tile([64, 512], F32,
                                  name=f"rb{p}{strip}{h01}", tag="rb")
                nc.gpsimd.partition_broadcast(rb[:], r[:])
                nc.vector.tensor_mul(
                    oht_tiles[p][h01 * 64:(h01 + 1) * 64, sl],
                    acc[0:64, :], rb[:])

        # strip 0: emit KT(p+1) BEFORE attention(p) so the scheduler can
        # hide the next K projection under the current pair's exp stream
        kt_tiles.append(proj_qk("k", wk, xk, bk_t, 0))
        for p in range(1, NP):
            kt_tiles.append(proj_qk("k", wk, xk, bk_t, p))
            attention(p - 1, 0)
        attention(NP - 1, 0)

        wo = load_w("wot", D)

        # strip 1 attention, strip-0 output projections emitted ahead of the
        # attention block they should overlap with
        for p in range(NP):
            if p < 4:
                o_proj(p)
            attention(p, 1)
        for stt in range(4, 8):
            o_proj(stt)
        if dbg is not None:
            for p in range(NP):
                nc.sync.dma_start(
                    dbg["doht"][p * 128:(p + 1) * 128, :],
                    oht_tiles[p][:].bitcast(F32))


def _get_nc():
    if "nc" not in _CACHE:
        _CACHE["nc"] = _build_nc()
    return _CACHE["nc"]


def kernel(queries, keys, values, Wq, bq, Wk, bk, Wv, bv, Wo, bo):
    queries = np.ascontiguousarray(queries, dtype=np.float32)
    keys = np.ascontiguousarray(keys, dtype=np.float32)
    values = np.ascontiguousarray(values, dtype=np.float32)

    wqt = np.ascontiguousarray(np.asarray(Wq, np.float32).T)
    wkt = np.ascontiguousarray(np.asarray(Wk, np.float32).T)
    wot = np.ascontiguousarray(np.asarray(Wo, np.float32).T)
    wvt = np.asarray(Wv, np.float32).T              # (D, D) = (di, do)
    wvtp = np.zeros((D, DVP), np.float32)
    for h in range(H):
        wvtp[:, h * (DH + 1):h * (DH + 1) + DH] = \
            wvt[:, h * DH:(h + 1) * DH]
    bo_eff = (np.asarray(bo, np.float32)
              + np.asarray(Wo, np.float32) @ np.asarray(bv, np.float32))
    bqc = np.ascontiguousarray(
        np.asarray(bq, np.float32).reshape(ND, 128).T)
    bkc = np.ascontiguousarray(
        np.asarray(bk, np.float32).reshape(ND, 128).T)
    bor = np.ascontiguousarray(bo_eff.reshape(1, D))

    shared = {
        "wqt": wqt, "wkt": wkt, "wvtp": wvtp, "wot": wot,
        "bqc": bqc, "bkc": bkc, "bor": bor,
    }
    in_maps = []
    for b in range(B):
        in_maps.append({
            "xqt": np.ascontiguousarray(queries[b].T),
            "xkt": np.ascontiguousarray(keys[b].T),
            "xvt": np.ascontiguousarray(values[b].T),
            **shared,
        })

    nc = _get_nc()
    res = run_bass_kernel_spmd(nc, in_maps, core_ids=list(range(B)))
    return np.stack([res.results[b]["out"] for b in range(B)], axis=0)


def run_traced(inputs, tmpdir=None):
    """Profiled single run; returns BassKernelResults with exec_time_ns."""
    queries = np.ascontiguousarray(inputs["queries"], dtype=np.float32)
    keys = np.ascontiguousarray(inputs["keys"], dtype=np.float32)
    values = np.ascontiguousarray(inputs["values"], dtype=np.float32)
    Wq, bq = inputs["Wq"], inputs["bq"]
    Wk, bk = inputs["Wk"], inputs["bk"]
    Wv, bv = inputs["Wv"], inputs["bv"]
    Wo, bo = inputs["Wo"], inputs["bo"]
    wvt = np.asarray(Wv, np.float32).T
    wvtp = np.zeros((D, DVP), np.float32)
    for h in range(H):
        wvtp[:, h * (DH + 1):h * (DH + 1) + DH] = wvt[:, h * DH:(h + 1) * DH]
    bo_eff = (np.asarray(bo, np.float32)
              + np.asarray(Wo, np.float32) @ np.asarray(bv, np.float32))
    shared = {
        "wqt": np.ascontiguousarray(np.asarray(Wq, np.float32).T),
        "wkt": np.ascontiguousarray(np.asarray(Wk, np.float32).T),
        "wvtp": wvtp,
        "wot": np.ascontiguousarray(np.asarray(Wo, np.float32).T),
        "bqc": np.ascontiguousarray(np.asarray(bq, np.float32).reshape(ND, 128).T),
        "bkc": np.ascontiguousarray(np.asarray(bk, np.float32).reshape(ND, 128).T),
        "bor": np.ascontiguousarray(bo_eff.reshape(1, D)),
    }
    in_maps = [
        {"xqt": np.ascontiguousarray(queries[b].T),
         "xkt": np.ascontiguousarray(keys[b].T),
         "xvt": np.ascontiguousarray(values[b].T), **shared}
        for b in range(B)
    ]
    nc = _get_nc()
    return run_bass_kernel_spmd(nc, in_maps, core_ids=list(range(B)),
                                trace=True, tmpdir=tmpdir)
